# revision 1
# baseline (speedup 1.0000x reference)
"""AvgPool2d-as-Toeplitz kernel for Trainium2 (8 NeuronCores, SPMD).

Reference computes out = (enc_x * mask) @ W.T where W is the dense
Toeplitz matrix of conv2d with kernel ones(C,C,KH,KW)/(KH*KW) over the
flattened zero-padded input (C=16, KH=KW=2, stride 2, pad 1, H=W=32),
and mask zeroes the 1-pixel padding ring of each 34x34 channel image.

Structure exploited:
  W[(co,oi,oj), (ci,i,j)] = 0.25  iff  i in {2oi, 2oi+1} and j in {2oj, 2oj+1}
— independent of co, summed over every ci. Hence with x viewed as
[B, C, 34, 34] and the mask ring folded in structurally (pooling windows
simply never read the masked border rows/columns):

  out[b, co, oi, oj] = 0.25 * sum_ci sum_window x[b, ci, i, j]
       over i in {2oi, 2oi+1} ∩ [1,32],  j in {2oj, 2oj+1} ∩ [1,32]

i.e. one channel-summed 2x2/stride-2 pooled [17,17] map per batch,
replicated across the 16 output channels. ~2.4 MB of input instead of
the 342 MB dense weight + 2.4 MB mask.

Per-core plan (4 batches per core, batch-parallel across 8 cores),
raw bacc with manual semaphores:
  partitions = (b, ci) = 64, free = flattened 34x34 channel image.
  GPS : zero rows 0/33 of the column-pooled tile (masked rows), then
        E[(b,ci),(b2,co)] = 0.25*(b==b2) via memset(0.25) + 2
        affine_selects — all hidden under the input DMA.
  ACT : DMA image rows 0-16 (ring qActDynamicHW).
  SP  : DMA image rows 17-33 (ring qSyncDynamicHW).
  DVE : per DMA half: column-pair add (interior rows) + border-column
        copy, then the row-pair adds (uniform — masked border rows are
        the pre-zeroed ones); finally copy PSUM -> SBUF.
  PE  : psum[(b,co),(oi,oj)] = E.T @ a2  (ci-sum + 0.25 + co-broadcast)
  ACT : DMA out [4, 4624], wait for completion.
"""

import sys

import numpy as np

if "/opt/trn_rl_repo" not in sys.path:
    sys.path.insert(0, "/opt/trn_rl_repo")

B, C = 32, 16
HP = WP = 34
OH = OW = 17
IMG = HP * WP             # 1156
IN_DIM = C * IMG          # 18496
OUT_DIM = C * OH * OW     # 4624
N_CORES = 8
B_SH = B // N_CORES       # 4 batches per core
P = B_SH * C              # 64 partitions in use

_PROGRAM = None


def _build_program():
    import concourse.bacc as bacc
    import concourse.mybir as mybir

    f32 = mybir.dt.float32
    add = mybir.AluOpType.add
    nc = bacc.Bacc()

    x = nc.declare_dram_parameter("x", [B_SH, IN_DIM], f32, isOutput=False)
    out = nc.declare_dram_parameter("out", [B_SH, OUT_DIM], f32, isOutput=True)
    xv = x[:, :].rearrange("b (c f) -> (b c) f", c=C)   # [64, 1156]
    ov = out[:, :].rearrange("b (co s) -> (b co) s", co=C)

    with (
        nc.sbuf_tensor([P, IMG], f32) as xt,
        nc.sbuf_tensor([P, P], f32) as et,
        nc.sbuf_tensor([P, HP * OW], f32) as at,
        nc.sbuf_tensor([P, OH * OW], f32) as a2t,
        nc.sbuf_tensor([P, OH * OW], f32) as ot,
        nc.psum_tensor([P, OH * OW], f32) as pt,
        nc.semaphore("s_dma0") as s_dma0,
        nc.semaphore("s_dma1") as s_dma1,
        nc.semaphore("s_gps") as s_gps,
        nc.semaphore("s_dve") as s_dve,
        nc.semaphore("s_pe") as s_pe,
        nc.semaphore("s_out") as s_out,
        nc.Block() as block,
    ):
        x3 = xt[:].rearrange("p (i j) -> p i j", i=HP)
        a3 = at[:].rearrange("p (i oj) -> p i oj", i=HP)
        a23 = a2t[:].rearrange("p (oi oj) -> p oi oj", oi=OH)
        e3 = et[:].rearrange("p (qb qc) -> p qb qc", qb=B_SH)

        RS = 17  # image-row split between the two HWDGE rings

        @block.scalar
        def _(scalar):
            # rows 0-16 on the ACT ring
            scalar.dma_start(xt[:, 0:RS * WP], xv[:, 0:RS * WP]).then_inc(
                s_dma0, 16
            )
            # out DMA once the DVE's PSUM->SBUF copy is done
            scalar.wait_ge(s_dve, 7)
            scalar.dma_start(ov[:], ot[:]).then_inc(s_out, 16)
            scalar.wait_ge(s_out, 16)

        @block.sync
        def _(sync):
            # rows 17-33 on the SP ring
            sync.dma_start(xt[:, RS * WP:IMG], xv[:, RS * WP:IMG]).then_inc(
                s_dma1, 16
            )

        @block.gpsimd
        def _(gpsimd):
            # masked image rows 0 and 33 of the column-pooled tile -> 0,
            # so the row-pair stage needs no border special-casing
            gpsimd.memset(a3[:, 0:HP:HP - 1, :], 0.0).then_inc(s_gps, 1)
            # E[p,(qb,qc)] = 0.25 iff 0 <= p - 16*qb <= 15
            gpsimd.memset(et[:], 0.25).then_inc(s_gps, 1)
            gpsimd.wait_ge(s_gps, 2)
            nc.gpsimd.affine_select(
                e3, e3, [[-C, B_SH], [0, C]], mybir.AluOpType.is_ge, 0.0,
                base=0, channel_multiplier=1,
            ).then_inc(s_gps, 1)
            gpsimd.wait_ge(s_gps, 3)
            nc.gpsimd.affine_select(
                e3, e3, [[C, B_SH], [0, C]], mybir.AluOpType.is_ge, 0.0,
                base=C - 1, channel_multiplier=-1,
            ).then_inc(s_gps, 1)

        @block.vector
        def _(vector):
            def ctt(r0, r1):
                return nc.vector.tensor_tensor(
                    a3[:, r0:r1, 1:16],
                    x3[:, r0:r1, 2:32:2], x3[:, r0:r1, 3:33:2], add,
                )

            def cb(r0, r1):
                return nc.vector.tensor_copy(
                    a3[:, r0:r1, 0:17:16], x3[:, r0:r1, 1:33:31]
                )

            vector.wait_ge(s_gps, 1)
            vector.wait_ge(s_dma0, 16)
            ctt(1, RS).then_inc(s_dve, 1)                 # rows 1-16
            cb(1, RS).then_inc(s_dve, 1)
            # a2 rows oi 0..7 from a rows 0..15 (row 0 pre-zeroed by GPS)
            nc.vector.tensor_tensor(
                a23[:, 0:8, :], a3[:, 0:16:2, :], a3[:, 1:17:2, :], add,
            ).then_inc(s_dve, 1)._wait_ge(s_dve, 2)
            vector.wait_ge(s_dma1, 16)
            ctt(RS, HP - 1).then_inc(s_dve, 1)            # rows 17-32
            cb(RS, HP - 1).then_inc(s_dve, 1)
            # a2 rows oi 8..16 from a rows 16..33 (row 33 pre-zeroed)
            nc.vector.tensor_tensor(
                a23[:, 8:17, :], a3[:, 16:34:2, :], a3[:, 17:34:2, :], add,
            ).then_inc(s_dve, 1)._wait_ge(s_dve, 5)
            vector.wait_ge(s_pe, 1)
            nc.vector.tensor_copy(ot[:], pt[:]).then_inc(s_dve, 1)

        @block.tensor
        def _(tensor):
            tensor.wait_ge(s_dve, 6)
            tensor.wait_ge(s_gps, 4)
            nc.tensor.matmul(
                pt[:], et[:], a2t[:], start=True, stop=True
            ).then_inc(s_pe, 1)

    nc.compile()
    return nc


def _get_program():
    global _PROGRAM
    if _PROGRAM is None:
        _PROGRAM = _build_program()
    return _PROGRAM


def _run(enc_x: np.ndarray, mask: np.ndarray = None, **spmd_kwargs):
    from concourse.bass_utils import run_bass_kernel_spmd

    nc = _get_program()
    in_maps = []
    for i in range(N_CORES):
        sl = slice(i * B_SH, (i + 1) * B_SH)
        in_maps.append({"x": np.ascontiguousarray(enc_x[sl], dtype=np.float32)})
    res = run_bass_kernel_spmd(nc, in_maps, list(range(N_CORES)), **spmd_kwargs)
    out = np.concatenate([res.results[i]["out"] for i in range(N_CORES)], axis=0)
    return out, res


def kernel(enc_x, weight=None, mask=None, **_unused):
    enc_x = np.asarray(enc_x, dtype=np.float32)
    assert enc_x.shape == (B, IN_DIM), enc_x.shape
    out, _ = _run(enc_x)
    return out

